# revision 1
# baseline (speedup 1.0000x reference)
"""KroneckerLinear Trainium2 kernel.

Math: out = x @ kron(f1, f2).T + bias, with x [64, 8192], f1 [128,128],
f2 [64,64], bias [8192].  Kronecker identity:
    out[b].reshape(128, 64) = f1 @ X_b @ f2.T,   X_b = x[b].reshape(128, 64)
so the 8192x8192 weight (256 MB) is never materialized; the kernel is
memory-bound on x in / out (~4 MB total).

Sharding: batch-parallel over the 8 NeuronCores, 8 batch rows per core.

Per-core device program (SPMD, identical on all cores), pipelined in 4
column slices p = 0..3:
  stage 1 (apply f2): matmul p has lhsT = xt[:, p*128:(p+1)*128] where xt
     is a host-prepared [128, 512] tile: rows (h*64+l), cols (p*128+j)
     hold x[lb, j*64+l] for local batch lb = p + 4h.  rhs =
     blkdiag(f2.T, f2.T) [128,128]: one K=128 matmul computes TWO
     batches: out[j, h*64+k] = (X_{p+4h} @ f2.T)[j, k].  Each slice gets
     its own PSUM bank so the stage-2 pipeline never bank-conflicts.
  stage 2 (apply f1): per slice: DVE copy V slice to SBUF, matmul
     lhsT = f1.T, rhs = V slice -> Y slice [i, (g, k)], g in {2p, 2p+1},
     local batch(g) = g//2 + 4*(g%2).
  bias: per slice, Y + bias.reshape(128, 64) broadcast over both groups
     (fused with the PSUM->SBUF move).
  store: per slice, 3-D DMA to y rows (2p, 2p+1); host unpermutes rows.
"""

import numpy as np

N_CORES = 8
B = 64
LB = B // N_CORES  # 8 local batches per core

_CACHE = {}


def _build_nc(use_f32r=False):
    import concourse.bass as bass
    import concourse.mybir as mybir
    import concourse.tile as tile
    from concourse import bacc

    fp32 = mybir.dt.float32
    mmdt = mybir.dt.float32r if use_f32r else fp32

    nc = bacc.Bacc("TRN2", target_bir_lowering=False, debug=False)
    # all inputs packed into one [128, 832] tensor:
    # blk 0:128 | f1t 128:256 | xt_p0 256:384 | biasr 384:448 | xt_p123 448:832
    in_d = nc.dram_tensor("inp", [128, 832], fp32, kind="ExternalInput")
    y_d = nc.dram_tensor("y", [LB, 8192], fp32, kind="ExternalOutput")

    with tile.TileContext(nc) as tc:
        with (
            tc.tile_pool(name="sb", bufs=1) as sb,
            tc.tile_pool(name="slc", bufs=4) as slc,
            tc.tile_pool(name="osb", bufs=2) as osb,
            tc.tile_pool(name="psv", bufs=4, space="PSUM") as psv,
            tc.tile_pool(name="psy", bufs=4, space="PSUM") as psy,
        ):
            inp = sb.tile([128, 832], fp32)
            blk = inp[:, 0:128]
            f1t = inp[:, 128:256]
            biasr = inp[:, 384:448]

            def xt_slice(p):
                return inp[:, 256:384] if p == 0 else inp[:, 320 + p * 128 : 448 + p * 128]

            # DMA 1: everything slice-0 compute needs; DMA 2: the rest.
            nc.sync.dma_start(out=inp[:, 0:384], in_=in_d[:, 0:384])
            nc.sync.dma_start(out=inp[:, 384:832], in_=in_d[:, 384:832])

            b_ap = biasr
            bias_bcast = bass.AP(
                tensor=b_ap.tensor,
                offset=b_ap.offset,
                ap=[b_ap.ap[0], [0, 2], b_ap.ap[1]],
            )

            def mm(ap):
                return ap.bitcast(mmdt) if use_f32r else ap

            out_halves = []
            for _h in range(2):
                out_half = osb.tile([128, 256], fp32, tag="out_sb")
                out_halves.append(out_half)

            v_all = sb.tile([128, 512], fp32)
            for p in range(4):
                psum_v = psv.tile([128, 128], fp32, tag="psum_v")
                nc.tensor.matmul(
                    psum_v[:], mm(xt_slice(p)), mm(blk), start=True, stop=True
                )
                # V copy on ACT (DVE is the busier engine: it owns the adds)
                nc.scalar.copy(v_all[:, p * 128 : (p + 1) * 128], psum_v[:])

            # stage 2: one N=512 matmul (f32r runs this at full rate)
            psum_y = psy.tile([128, 512], fp32)
            nc.tensor.matmul(psum_y[:], mm(f1t), mm(v_all[:, :]), start=True, stop=True)

            for p in range(4):
                out_sb = out_halves[p // 2]
                o_ap = out_sb[:, (p % 2) * 128 : (p % 2) * 128 + 128]
                out_g = bass.AP(
                    tensor=o_ap.tensor,
                    offset=o_ap.offset,
                    ap=[o_ap.ap[0], [64, 2], [1, 64]],
                )
                y_ap = psum_y[:, p * 128 : (p + 1) * 128]
                y_g = bass.AP(
                    tensor=y_ap.tensor,
                    offset=y_ap.offset,
                    ap=[y_ap.ap[0], [64, 2], [1, 64]],
                )
                nc.vector.tensor_add(out_g, y_g, bias_bcast)

                if p % 2 == 1:
                    # store half -> y rows (2p-2 .. 2p+1) in group order
                    s_ap = out_sb[:, :]
                    src = bass.AP(
                        tensor=s_ap.tensor,
                        offset=s_ap.offset,
                        ap=[s_ap.ap[0], [64, 4], [1, 64]],
                    )
                    d_ap = y_d[:, :]
                    dst = bass.AP(
                        tensor=d_ap.tensor,
                        offset=d_ap.offset + (2 * p - 2) * 8192,
                        ap=[[64, 128], [8192, 4], [1, 64]],
                    )
                    # SP is free after the input loads
                    nc.sync.dma_start(out=dst, in_=src)

    nc.compile()
    return nc


def _prep_core_inputs(x, factor1, factor2, bias):
    """Host-side layout prep. Returns list of per-core in_maps."""
    x = np.ascontiguousarray(np.asarray(x, dtype=np.float32))
    f1 = np.asarray(factor1, dtype=np.float32)
    f2 = np.asarray(factor2, dtype=np.float32)
    bias = np.asarray(bias, dtype=np.float32)

    # x -> per-core xt [128, 512]: xt[h*64+l, p*128+j] = x[c*8 + p + 4h, j*64+l]
    xc = x.reshape(N_CORES, LB, 128, 64)  # [c, lb, j, l]
    arr = xc.transpose(0, 3, 1, 2).reshape(N_CORES, 64, 2, 4, 128)
    xt_all = arr.transpose(0, 2, 1, 3, 4).reshape(N_CORES, 128, 512)

    # packed input [128, 832]: blk | f1t | xt_p0 | biasr | xt_p123
    inp_all = np.zeros((N_CORES, 128, 832), dtype=np.float32)
    inp_all[:, :64, 0:64] = f2.T
    inp_all[:, 64:, 64:128] = f2.T
    inp_all[:, :, 128:256] = f1.T
    inp_all[:, :, 256:384] = xt_all[:, :, 0:128]
    inp_all[:, :, 384:448] = bias.reshape(128, 64)
    inp_all[:, :, 448:832] = xt_all[:, :, 128:512]

    return [{"inp": np.ascontiguousarray(inp_all[c])} for c in range(N_CORES)]


def kernel(x, factor1, factor2, bias):
    from concourse.bass_utils import run_bass_kernel_spmd

    if "nc" not in _CACHE:
        _CACHE["nc"] = _build_nc()
    nc = _CACHE["nc"]

    in_maps = _prep_core_inputs(x, factor1, factor2, bias)
    res = run_bass_kernel_spmd(nc, in_maps, core_ids=list(range(N_CORES)))
    kernel.last_results = res

    # device writes y rows in group order g (batch = g//2 + 4*(g%2));
    # unpermute to batch order: inv = argsort([0,4,1,5,2,6,3,7])
    inv = np.array([0, 2, 4, 6, 1, 3, 5, 7])
    out = np.concatenate(
        [res.results[c]["y"][inv] for c in range(N_CORES)], axis=0
    )
    return out



# revision 2
# speedup vs baseline: 1.3745x; 1.3745x over previous
"""KroneckerLinear Trainium2 kernel (bf16 data path).

Math: out = x @ kron(f1, f2).T + bias, with x [64, 8192], f1 [128,128],
f2 [64,64], bias [8192].  Kronecker identity:
    out[b].reshape(128, 64) = f1 @ X_b @ f2.T,   X_b = x[b].reshape(128, 64)
so the 8192x8192 weight (256 MB) is never materialized; the kernel is
memory-bound on x in / out.

Sharding: batch-parallel over the 8 NeuronCores, 8 batch rows per core.

All device I/O is bf16 (tolerance is 2e-2; bf16 end-to-end error is ~1e-3):
halves DMA bytes and runs the PE at full bf16 rate instead of 1/4-rate fp32.

Per-core device program (SPMD, identical on all cores):
  input: one packed [128, 832] bf16 tensor
     cols [0:128] blk = blkdiag(f2.T, f2.T), [128:640] xt, [640:768] f1t,
     [768:832] bias.reshape(128, 64).
     xt[h*64+l, p*128+j] = x[c*8 + p + 4h, j*64+l]  (local batch lb = p+4h)
  stage 1 (apply f2), slice p = 0..3: matmul lhsT = xt_p, rhs = blk ->
     psum_v_p[j, h*64+k] = (X_{p+4h} @ f2.T)[j, k]; ACT copies to bf16 SBUF.
  stage 2 (apply f1), slice p: matmul lhsT = f1t, rhs = v_p ->
     psum_y_p[i, h*64+k] = (f1 @ X_{p+4h} @ f2.T)[i, k]
  bias+store: DVE adds bias (broadcast over h) casting to bf16, then a
     per-slice DMA stores to y[:, p*128:(p+1)*128]; host unpermutes.
"""

import numpy as np

N_CORES = 8
B = 64
LB = B // N_CORES  # 8 local batches per core

_CACHE = {}


def _build_nc():
    import concourse.bass as bass
    import concourse.mybir as mybir
    import concourse.tile as tile
    from concourse import bacc

    fp32 = mybir.dt.float32
    bf16 = mybir.dt.bfloat16

    nc = bacc.Bacc("TRN2", target_bir_lowering=False, debug=False)
    in_d = nc.dram_tensor("inp", [128, 832], bf16, kind="ExternalInput")
    y_d = nc.dram_tensor("y", [128, 512], bf16, kind="ExternalOutput")

    with tile.TileContext(nc) as tc:
        with (
            tc.tile_pool(name="sb", bufs=1) as sb,
            tc.tile_pool(name="psv", bufs=4, space="PSUM") as psv,
            tc.tile_pool(name="psy", bufs=4, space="PSUM") as psy,
        ):
            inp = sb.tile([128, 832], bf16)
            blk = inp[:, 0:128]
            f1t = inp[:, 640:768]
            biasr = inp[:, 768:832]

            def xt_slice(p):
                return inp[:, 128 + p * 128 : 256 + p * 128]

            # DMA 1: blk + xt (all stage-1 deps); DMA 2: f1t + bias.
            nc.sync.dma_start(out=inp[:, 0:640], in_=in_d[:, 0:640])
            nc.sync.dma_start(out=inp[:, 640:832], in_=in_d[:, 640:832])

            b_ap = biasr
            bias_bcast = bass.AP(
                tensor=b_ap.tensor,
                offset=b_ap.offset,
                ap=[b_ap.ap[0], [0, 2], b_ap.ap[1]],
            )

            def grouped(ap):
                # [128, 128] -> [128, 2, 64] view (h group, k)
                return bass.AP(
                    tensor=ap.tensor,
                    offset=ap.offset,
                    ap=[ap.ap[0], [64, 2], [1, 64]],
                )

            v_sb = sb.tile([128, 512], bf16)
            out_sb = sb.tile([128, 512], bf16)

            psum_vs = []
            for p in range(4):
                psum_v = psv.tile([128, 128], fp32, tag="psum_v")
                nc.tensor.matmul(
                    psum_v[:], xt_slice(p), blk, start=True, stop=True
                )
                # V copy on ACT (cast fp32 -> bf16); DVE owns the adds
                nc.scalar.copy(v_sb[:, p * 128 : (p + 1) * 128], psum_v[:])
                psum_vs.append(psum_v)

            for p in range(4):
                psum_y = psy.tile([128, 128], fp32, tag="psum_y")
                nc.tensor.matmul(
                    psum_y[:],
                    f1t,
                    v_sb[:, p * 128 : (p + 1) * 128],
                    start=True,
                    stop=True,
                )
                o_ap = out_sb[:, p * 128 : (p + 1) * 128]
                nc.vector.tensor_add(grouped(o_ap), grouped(psum_y[:]), bias_bcast)
                nc.sync.dma_start(
                    out=y_d[:, p * 128 : (p + 1) * 128], in_=o_ap
                )

    nc.compile()
    return nc


def _prep_core_inputs(x, factor1, factor2, bias):
    """Host-side layout prep. Returns list of per-core in_maps."""
    import ml_dtypes

    bf16 = ml_dtypes.bfloat16
    x = np.ascontiguousarray(np.asarray(x, dtype=np.float32))
    f1 = np.asarray(factor1, dtype=np.float32)
    f2 = np.asarray(factor2, dtype=np.float32)
    bias = np.asarray(bias, dtype=np.float32)

    # x -> per-core xt [128, 512]: xt[h*64+l, p*128+j] = x[c*8 + p + 4h, j*64+l]
    xc = x.reshape(N_CORES, LB, 128, 64)  # [c, lb, j, l]
    arr = xc.transpose(0, 3, 1, 2).reshape(N_CORES, 64, 2, 4, 128)  # [c, l, h, p, j]
    xt_all = arr.transpose(0, 2, 1, 3, 4).reshape(N_CORES, 128, 512)  # [c, (h,l), (p,j)]

    # packed input [128, 832] bf16: blk | xt | f1t | biasr
    inp_all = np.zeros((N_CORES, 128, 832), dtype=np.float32)
    inp_all[:, :64, 0:64] = f2.T
    inp_all[:, 64:, 64:128] = f2.T
    inp_all[:, :, 128:640] = xt_all
    inp_all[:, :, 640:768] = f1.T
    inp_all[:, :, 768:832] = bias.reshape(128, 64)
    inp_all = inp_all.astype(bf16)

    return [{"inp": np.ascontiguousarray(inp_all[c])} for c in range(N_CORES)]


def kernel(x, factor1, factor2, bias):
    from concourse.bass_utils import run_bass_kernel_spmd

    if "nc" not in _CACHE:
        _CACHE["nc"] = _build_nc()
    nc = _CACHE["nc"]

    in_maps = _prep_core_inputs(x, factor1, factor2, bias)
    res = run_bass_kernel_spmd(nc, in_maps, core_ids=list(range(N_CORES)))
    kernel.last_results = res

    # y[i, p*128 + h*64 + k] = out[c*8 + p + 4h, i*64 + k]
    outs = []
    for c in range(N_CORES):
        y = np.asarray(res.results[c]["y"], dtype=np.float32)
        yr = y.reshape(128, 4, 2, 64).transpose(2, 1, 0, 3).reshape(LB, 8192)
        outs.append(yr)
    return np.concatenate(outs, axis=0)


# revision 4
# speedup vs baseline: 1.5381x; 1.1190x over previous
"""KroneckerLinear Trainium2 kernel (bf16 data path).

Math: out = x @ kron(f1, f2).T + bias, with x [64, 8192], f1 [128,128],
f2 [64,64], bias [8192].  Kronecker identity:
    out[b].reshape(128, 64) = f1 @ X_b @ f2.T,   X_b = x[b].reshape(128, 64)
so the 8192x8192 weight (256 MB) is never materialized; the kernel is
memory-bound on x in / out.

Sharding: batch-parallel over the 8 NeuronCores, 8 batch rows per core.

All device I/O is bf16 (tolerance is 2e-2; bf16 end-to-end error is ~1e-3):
halves DMA bytes and runs the PE at full bf16 rate instead of 1/4-rate fp32.

Per-core device program (SPMD, identical on all cores):
  input: one packed [128, 832] bf16 tensor
     cols [0:128] blk = blkdiag(f2.T, f2.T), [128:640] xt, [640:768] f1t,
     [768:832] bias.reshape(128, 64).
     xt[h*64+l, p*128+j] = x[c*8 + p + 4h, j*64+l]  (local batch lb = p+4h)
  stage 1 (apply f2), slice p = 0..3: matmul lhsT = xt_p, rhs = blk ->
     psum_v_p[j, h*64+k] = (X_{p+4h} @ f2.T)[j, k]; ACT copies to bf16 SBUF.
  stage 2 (apply f1), slice p: matmul lhsT = f1t, rhs = v_p ->
     psum_y_p[i, h*64+k] = (f1 @ X_{p+4h} @ f2.T)[i, k]
  bias+store: DVE adds bias (broadcast over h) casting to bf16, then a
     per-slice DMA stores to y[:, p*128:(p+1)*128]; host unpermutes.
"""

import numpy as np

N_CORES = 8
B = 64
LB = B // N_CORES  # 8 local batches per core

_CACHE = {}


def _build_nc():
    import concourse.bass as bass
    import concourse.mybir as mybir
    import concourse.tile as tile
    from concourse import bacc

    fp32 = mybir.dt.float32
    bf16 = mybir.dt.bfloat16

    nc = bacc.Bacc("TRN2", target_bir_lowering=False, debug=False)
    in_d = nc.dram_tensor("inp", [128, 832], bf16, kind="ExternalInput")
    y_d = nc.dram_tensor("y", [128, 512], bf16, kind="ExternalOutput")

    with tile.TileContext(nc) as tc:
        with (
            tc.tile_pool(name="sb", bufs=1) as sb,
            tc.tile_pool(name="psv", bufs=4, space="PSUM") as psv,
            tc.tile_pool(name="psy", bufs=4, space="PSUM") as psy,
        ):
            inp = sb.tile([128, 832], bf16)
            blk = inp[:, 0:128]
            f1t = inp[:, 640:768]
            biasr = inp[:, 768:832]

            def xt_slice(p):
                return inp[:, 128 + p * 128 : 256 + p * 128]

            # DMA 1: blk + xt (all stage-1 deps); DMA 2: f1t + bias.
            nc.sync.dma_start(out=inp[:, 0:640], in_=in_d[:, 0:640])
            nc.sync.dma_start(out=inp[:, 640:832], in_=in_d[:, 640:832])

            b_ap = biasr
            bias_bcast = bass.AP(
                tensor=b_ap.tensor,
                offset=b_ap.offset,
                ap=[b_ap.ap[0], [0, 2], b_ap.ap[1]],
            )

            def grouped(ap):
                # [128, 128] -> [128, 2, 64] view (h group, k)
                return bass.AP(
                    tensor=ap.tensor,
                    offset=ap.offset,
                    ap=[ap.ap[0], [64, 2], [1, 64]],
                )

            v_sb = sb.tile([128, 512], bf16)
            out_sb = sb.tile([128, 512], bf16)

            for p in range(4):
                psum_v = psv.tile([128, 128], fp32, tag="psum_v")
                nc.tensor.matmul(
                    psum_v[:], xt_slice(p), blk, start=True, stop=True
                )
                # split the PSUM->SBUF copies across ACT and DVE so they
                # don't serialize on one engine (they gate stage 2)
                if p % 2 == 0:
                    nc.scalar.copy(v_sb[:, p * 128 : (p + 1) * 128], psum_v[:])
                else:
                    nc.vector.tensor_copy(v_sb[:, p * 128 : (p + 1) * 128], psum_v[:])

            for p in range(4):
                psum_y = psy.tile([128, 128], fp32, tag="psum_y")
                nc.tensor.matmul(
                    psum_y[:],
                    f1t,
                    v_sb[:, p * 128 : (p + 1) * 128],
                    start=True,
                    stop=True,
                )
                o_ap = out_sb[:, p * 128 : (p + 1) * 128]
                nc.vector.tensor_add(grouped(o_ap), grouped(psum_y[:]), bias_bcast)
                if p == 1:
                    # first half store: SP's HWDGE ring
                    nc.sync.dma_start(out=y_d[:, 0:256], in_=out_sb[:, 0:256])
                elif p == 3:
                    # second half store: ACT's HWDGE ring (parallel desc-gen)
                    nc.scalar.dma_start(out=y_d[:, 256:512], in_=out_sb[:, 256:512])

    nc.compile()
    return nc


def _prep_core_inputs(x, factor1, factor2, bias):
    """Host-side layout prep. Returns list of per-core in_maps."""
    import ml_dtypes

    bf16 = ml_dtypes.bfloat16
    x = np.ascontiguousarray(np.asarray(x, dtype=np.float32))
    f1 = np.asarray(factor1, dtype=np.float32)
    f2 = np.asarray(factor2, dtype=np.float32)
    bias = np.asarray(bias, dtype=np.float32)

    # x -> per-core xt [128, 512]: xt[h*64+l, p*128+j] = x[c*8 + p + 4h, j*64+l]
    xc = x.reshape(N_CORES, LB, 128, 64)  # [c, lb, j, l]
    arr = xc.transpose(0, 3, 1, 2).reshape(N_CORES, 64, 2, 4, 128)  # [c, l, h, p, j]
    xt_all = arr.transpose(0, 2, 1, 3, 4).reshape(N_CORES, 128, 512)  # [c, (h,l), (p,j)]

    # packed input [128, 832] bf16: blk | xt | f1t | biasr
    inp_all = np.zeros((N_CORES, 128, 832), dtype=np.float32)
    inp_all[:, :64, 0:64] = f2.T
    inp_all[:, 64:, 64:128] = f2.T
    inp_all[:, :, 128:640] = xt_all
    inp_all[:, :, 640:768] = f1.T
    inp_all[:, :, 768:832] = bias.reshape(128, 64)
    inp_all = inp_all.astype(bf16)

    return [{"inp": np.ascontiguousarray(inp_all[c])} for c in range(N_CORES)]


def kernel(x, factor1, factor2, bias):
    from concourse.bass_utils import run_bass_kernel_spmd

    if "nc" not in _CACHE:
        _CACHE["nc"] = _build_nc()
    nc = _CACHE["nc"]

    in_maps = _prep_core_inputs(x, factor1, factor2, bias)
    res = run_bass_kernel_spmd(nc, in_maps, core_ids=list(range(N_CORES)))
    kernel.last_results = res

    # y[i, p*128 + h*64 + k] = out[c*8 + p + 4h, i*64 + k]
    outs = []
    for c in range(N_CORES):
        y = np.asarray(res.results[c]["y"], dtype=np.float32)
        yr = y.reshape(128, 4, 2, 64).transpose(2, 1, 0, 3).reshape(LB, 8192)
        outs.append(yr)
    return np.concatenate(outs, axis=0)
